# revision 1
# baseline (speedup 1.0000x reference)
"""ClassAttention kernel for 8x TRN2 NeuronCores (Bass/Tile).

Problem (hardcoded): x[16, 2049, 1024], w_qkv[3072, 1024], w_proj[1024, 1024],
b_proj[1024].  Reference computes qkv projection, class-token attention
(only query position 0 attends), projection of the class token, and returns
concat([cls_tok, x[:, 1:]], axis=1).

Only output row 0 is computed; rows 1.. are x passthrough (done on host at
gather time, mirroring the reference's concatenate).

Algebraic restructure (exact same math, far fewer FLOPs):
    q0[b]        = x[b,0] @ Wq^T                        (host, tiny)
    wfold[b,h,:] = SCALE * q0[b,h,:] @ Wk_h             (host: fold q0 into Wk)
    logits[b,h,s]= sum_d x[b,s,d] * wfold[b,h,d]        (device matmul over d)
    attn         = softmax_s(logits)                    (device)
    xa[b,h,d]    = sum_s attn[b,h,s] * x[b,s,d]         (device matmul over s)
    cls2[b,g,he] = sum_d xa[b,g,d] * WvT[d,he]          (device, dense; the
                   needed cls[b,he] is the diagonal block g = he//64)
    out0[b,f]    = sum_d cls[b,d] * WpT[d,f] + bp[f]    (device)

Sharding: data-parallel over batch, 2 batch elements per core (8 cores).
x is shipped in both natural [s,d] (bf16) and transposed [d,s] (fp8 e3m4)
layouts so both contractions stream with the contraction on the partition dim.
"""

import os
import numpy as np
import ml_dtypes

BF16 = ml_dtypes.bfloat16
FP8 = ml_dtypes.float8_e3m4

# dtype knobs for the two big x streams (bfloat16 | float8e3)
XT_DTYPE = os.environ.get("K_XT_DTYPE", "float8e3")
XN_DTYPE = os.environ.get("K_XN_DTYPE", "float8e3")
_NP_OF = {"bfloat16": BF16, "float8e3": FP8}

B, S, D, H, E = 16, 2049, 1024, 16, 64
SCALE = E ** -0.5
NCORES = 8
BL = B // NCORES          # batches per core = 2
ST = 17                   # s-tiles of 128 (padded)
SP = ST * 128             # 2176 padded sequence
DT = 8                    # d-tiles of 128
NEG_BIG = -30000.0

_cached = {}


def _kernel_body(ctx, tc):
    import concourse.bass as bass
    from concourse import mybir

    nc = tc.nc
    dt = mybir.dt
    AF = mybir.ActivationFunctionType

    xt_dt = getattr(dt, XT_DTYPE)
    xn_dt = getattr(dt, XN_DTYPE)
    xn_d = nc.dram_tensor("xn", (BL * SP, D), xn_dt, kind="ExternalInput").ap()
    xt_d = nc.dram_tensor("xt", (BL * D, S), xt_dt, kind="ExternalInput").ap()
    wf_d = nc.dram_tensor("wf", (128, BL * 128), dt.bfloat16, kind="ExternalInput").ap()
    wv_d = nc.dram_tensor("wv", (D, D), dt.bfloat16, kind="ExternalInput").ap()
    wp_d = nc.dram_tensor("wp", (D, D), dt.bfloat16, kind="ExternalInput").ap()
    bp_d = nc.dram_tensor("bp", (BL, D), dt.float32, kind="ExternalInput").ap()
    id_d = nc.dram_tensor("ident", (48, 48), dt.bfloat16, kind="ExternalInput").ap()
    out_d = nc.dram_tensor("out", (BL, D), dt.float32, kind="ExternalOutput").ap()

    cpool = ctx.enter_context(tc.tile_pool(name="const", bufs=1))
    xn_pool = ctx.enter_context(tc.tile_pool(name="xn", bufs=1))
    xt_pool = ctx.enter_context(tc.tile_pool(name="xt", bufs=4))
    w_pool = ctx.enter_context(tc.tile_pool(name="w", bufs=1))
    sm_pool = ctx.enter_context(tc.tile_pool(name="sm", bufs=1))
    st_pool = ctx.enter_context(tc.tile_pool(name="stats", bufs=2))
    at_pool = ctx.enter_context(tc.tile_pool(name="attnT", bufs=2))
    acc_pool = ctx.enter_context(tc.tile_pool(name="acc", bufs=1))

    # PSUM: c0..c4 (5 banks, time-shared), tr (2 banks), xa (1 bank)
    ps_log = ctx.enter_context(tc.tile_pool(name="pslog", bufs=1, space="PSUM"))
    ps_tr = ctx.enter_context(tc.tile_pool(name="pstr", bufs=2, space="PSUM"))
    ps_xa = ctx.enter_context(tc.tile_pool(name="psxa", bufs=1, space="PSUM"))

    # --- constants ---
    wf_sb = cpool.tile([128, BL * 128], dt.bfloat16, tag="wf")
    nc.sync.dma_start(wf_sb[:], wf_d)
    id_sb = cpool.tile([48, 48], dt.bfloat16, tag="ident")
    nc.sync.dma_start(id_sb[:], id_d)
    bp_sb = cpool.tile([BL, D], dt.float32, tag="bp")
    nc.sync.dma_start(bp_sb[:], bp_d)

    xn_sb = [xn_pool.tile([128, ST * 1024], xn_dt, tag=f"xn{b}", name=f"xn{b}")
             for b in range(BL)]
    wv_sb = w_pool.tile([128, DT * 1024], dt.bfloat16, tag="wv")
    wp_sb = w_pool.tile([128, DT * 1024], dt.bfloat16, tag="wp")

    def load_xn(b, st0, st1):
        src = xn_d[b * SP + st0 * 128: b * SP + st1 * 128, :]
        nc.sync.dma_start(
            xn_sb[b][:, st0 * 1024: st1 * 1024]
            .rearrange("p (st d) -> p st d", st=st1 - st0),
            src.rearrange("(st p) d -> p st d", p=128),
        )

    def load_w(t, src):
        nc.sync.dma_start(
            t[:].rearrange("p (k c) -> p k c", k=DT),
            src.rearrange("(k p) c -> p k c", p=128),
        )

    def load_xt(b, hh):
        t = xt_pool.tile([128, 4 * S], xt_dt, tag="xt", name=f"xt{b}_{hh}")
        r0 = b * D + hh * 512
        src = xt_d[r0:r0 + 512, :].rearrange("(k p) s -> p k s", p=128)
        nc.sync.dma_start(t[:].rearrange("p (k s) -> p k s", k=4), src)
        return t

    # persistent accumulators
    xaT_sb = [acc_pool.tile([128, DT * H], dt.bfloat16, tag=f"xaT{b}",
                            name=f"xaT{b}") for b in range(BL)]  # col=d8*16+g
    cls_sb = acc_pool.tile([128, DT * BL], dt.bfloat16, tag="clsT")  # col=dp*2+b
    out_sb = acc_pool.tile([BL, D], dt.float32, tag="out")

    # --- PE warm-up: dense zero matmuls so logits run at 2.4GHz ---
    warm_sb = cpool.tile([128, 512], dt.bfloat16, tag="warm")
    nc.vector.memset(warm_sb[:], 0.0)
    for w in range(12):
        ps = ps_tr.tile([128, 512], dt.float32, tag="tr", name=f"warm{w}")
        nc.tensor.matmul(ps[:], warm_sb[:, :128], warm_sb[:], start=True, stop=True)

    # --- DMA program order (= sync-queue FIFO order) ---
    xt_tiles = {}
    xt_tiles[(0, 0)] = load_xt(0, 0)
    xt_tiles[(0, 1)] = load_xt(0, 1)
    xt_tiles[(1, 0)] = load_xt(1, 0)
    xt_tiles[(1, 1)] = load_xt(1, 1)
    load_xn(0, 0, 6)
    load_xn(0, 6, 12)
    load_xn(0, 12, 17)
    load_w(wv_sb, wv_d)
    load_xn(1, 0, 6)
    load_xn(1, 6, 12)
    load_xn(1, 12, 17)
    load_w(wp_sb, wp_d)

    def emit_logits(b):
        halves = [xt_tiles[(b, 0)], xt_tiles[(b, 1)]]
        chunks = [ps_log.tile([16, 512], dt.float32, tag=f"c{sc}", name=f"c{sc}_{b}")
                  for sc in range(5)]
        for d8 in range(8):
            xtt = halves[d8 // 4]
            lhs = wf_sb[:, b * 128 + d8 * 16: b * 128 + (d8 + 1) * 16]
            base = (d8 % 4) * S
            for sc in range(5):
                n = 512 if sc < 4 else 1
                nc.tensor.matmul(
                    chunks[sc][:, :n], lhs, xtt[:, base + sc * 512: base + sc * 512 + n],
                    start=(d8 == 0), stop=(d8 == 7),
                )
        return chunks

    def emit_softmax_pre(b, chunks):
        # logits ~ N(0,1): exp() cannot overflow fp32, so skip the max-shift
        # entirely and exp straight out of PSUM with accumulated sums.
        expv = sm_pool.tile([16, SP], dt.float32, tag="exp", name=f"exp{b}", bufs=2)
        nc.vector.memset(expv[:, S:], 0.0)
        sums = st_pool.tile([16, 5], dt.float32, tag="sums", name=f"sums{b}")
        for sc in range(5):
            n = 512 if sc < 4 else 1
            nc.scalar.activation(expv[:, sc * 512: sc * 512 + n], chunks[sc][:, :n],
                                 AF.Exp, bias=0.0, scale=1.0,
                                 accum_out=sums[:, sc: sc + 1])
        return expv, sums

    def emit_softmax_post(b, expv, sums):
        sumexp = st_pool.tile([16, 1], dt.float32, tag="sumexp", name=f"sumexp{b}")
        nc.vector.tensor_reduce(
            sumexp[:], sums[:], axis=mybir.AxisListType.X, op=mybir.AluOpType.add)
        recip = st_pool.tile([16, 1], dt.float32, tag="recip", name=f"recip{b}")
        nc.vector.reciprocal(recip[:], sumexp[:])
        attn = sm_pool.tile([16, SP], dt.bfloat16, tag="attn", name=f"attn{b}", bufs=2)
        nc.vector.tensor_scalar_mul(attn[:], expv[:], recip[:])
        return attn

    def grouped_transposes(pfx, src_sb, n_tr, dst_sb, dst_col0):
        """Transpose [16,128] slices of src into [128,16] columns of dst,
        batching 4 per PSUM bank so one DVE copy retires 4 transposes."""
        for g0 in range(0, n_tr, 4):
            g1 = min(g0 + 4, n_tr)
            ps = ps_tr.tile([128, 64], dt.bfloat16, tag="tr", name=f"{pfx}_{g0}")
            for k in range(g0, g1):
                nc.tensor.transpose(ps[:, (k - g0) * 16:(k - g0 + 1) * 16],
                                    src_sb[:, k * 128:(k + 1) * 128],
                                    id_sb[:16, :16])
            nc.vector.tensor_copy(
                dst_sb[:, dst_col0 + g0 * 16: dst_col0 + g1 * 16],
                ps[:, :(g1 - g0) * 16])

    def emit_transposes(b, attn):
        atT = at_pool.tile([128, ST * 16], dt.bfloat16, tag="attnT", name=f"atT{b}")
        grouped_transposes(f"at{b}", attn, ST, atT, 0)
        return atT

    xa2 = acc_pool.tile([48, D], dt.bfloat16, tag="xa2")
    nc.vector.memset(xa2[:], 0.0)

    def emit_xa(b, atT, dual):
        # xa[h, d] = sum_s attn[h,s] x[s,d]: attnT stationary (16-col loads),
        # xn moving at N=512; accumulate the two 512-wide d-chunks.
        accs = [ps_xa.tile([16, 512], dt.float32, tag="xa", name=f"xac{b}_0")]
        if dual:
            accs.append(ps_tr.tile([16, 512], dt.float32, tag="tr", name=f"xac{b}_1"))
        nch = 2 if dual else 1
        for base in range(0, 2, nch):
            for st in range(ST):
                for j in range(nch):
                    c = base + j
                    nc.tensor.matmul(
                        accs[j][:],
                        atT[:, st * 16:(st + 1) * 16],
                        xn_sb[b][:, st * 1024 + c * 512: st * 1024 + (c + 1) * 512],
                        start=(st == 0), stop=(st == ST - 1),
                    )
            for j in range(nch):
                c = base + j
                nc.vector.tensor_copy(xa2[32 * b: 32 * b + 16, c * 512:(c + 1) * 512],
                                      accs[j][:])

    def emit_tail():
        # xaT2[d, (d8, b, g)]: 8 transposes of [48,128] cover both batches
        xaT2 = acc_pool.tile([128, DT * 32], dt.bfloat16, tag="xaT2")
        for g0 in (0, 4):
            ps = ps_tr.tile([128, 192], dt.bfloat16, tag="tr", name=f"xtr{g0}")
            for k in range(4):
                d8 = g0 + k
                nc.tensor.transpose(ps[:, k * 48:(k + 1) * 48],
                                    xa2[:, d8 * 128:(d8 + 1) * 128], id_sb[:])
            for k in range(4):
                d8 = g0 + k
                for b in range(BL):
                    nc.vector.tensor_copy(
                        xaT2[:, d8 * 32 + b * 16: d8 * 32 + b * 16 + 16],
                        ps[:, k * 48 + 32 * b: k * 48 + 32 * b + 16])
        # dense cls2 for BOTH batches: out rows = b*16+g
        c2ps = [ps_log.tile([32, 512], dt.float32, tag=f"c{c}", name=f"c2_{c}")
                for c in range(2)]
        for c in range(2):
            for d8 in range(8):
                nc.tensor.matmul(
                    c2ps[c][:],
                    xaT2[:, d8 * 32:(d8 + 1) * 32],
                    wv_sb[:, d8 * 1024 + c * 512: d8 * 1024 + (c + 1) * 512],
                    start=(d8 == 0), stop=(d8 == 7),
                )
        c2pk = sm_pool.tile([32, D], dt.bfloat16, tag="c2", name="c2pk")
        for c in range(2):
            nc.vector.tensor_copy(c2pk[:, c * 512:(c + 1) * 512], c2ps[c][:])
        # transpose + diagonal-block select:
        # c2T col = dp*32 + b*16 + g; need g=2dp (rows 0-63), 2dp+1 (rows 64-127)
        c2T = acc_pool.tile([128, DT * 32], dt.bfloat16, tag="c2T")
        for g0 in (0, 4):
            ps = ps_tr.tile([128, 128], dt.bfloat16, tag="tr", name=f"selt{g0}")
            for k in range(4):
                dp = g0 + k
                nc.tensor.transpose(ps[:, k * 32:(k + 1) * 32],
                                    c2pk[:, dp * 128:(dp + 1) * 128],
                                    id_sb[:32, :32])
            nc.vector.tensor_copy(c2T[:, g0 * 32:(g0 + 4) * 32], ps[:])
        for b in range(BL):
            nc.vector.tensor_copy(cls_sb[0:64, b: b + 15: 2],
                                  c2T[0:64, b * 16: b * 16 + 239: 34])
            nc.vector.tensor_copy(cls_sb[64:128, b: b + 15: 2],
                                  c2T[64:128, b * 16 + 1: b * 16 + 240: 34])

    def emit_cls(b, ctags):
        # dense cls2[g, he] = sum_d xa[g, d] wv[d, he]; the diagonal block is
        # selected after a transpose: clsT[he, b] = cls2T[he, g=he//64]
        c2ps = [ps_log.tile([16, 512], dt.float32, tag=ctags[c], name=f"c2_{b}_{c}")
                for c in range(2)]
        for c in range(2):
            for d8 in range(8):
                nc.tensor.matmul(
                    c2ps[c][:],
                    xaT_sb[b][:, d8 * 16:(d8 + 1) * 16],
                    wv_sb[:, d8 * 1024 + c * 512: d8 * 1024 + (c + 1) * 512],
                    start=(d8 == 0), stop=(d8 == 7),
                )
        c2sb = sm_pool.tile([16, D], dt.bfloat16, tag="c2", name=f"c2sb{b}", bufs=2)
        for c in range(2):
            nc.vector.tensor_copy(c2sb[:, c * 512:(c + 1) * 512], c2ps[c][:])
        for g0 in (0, 4):
            ps = ps_tr.tile([128, 64], dt.bfloat16, tag="tr", name=f"sel{b}_{g0}")
            for k in range(4):
                dp = g0 + k
                nc.tensor.transpose(ps[:, k * 16:(k + 1) * 16],
                                    c2sb[:, dp * 128:(dp + 1) * 128], id_sb[:])
            # in-cols k*16 + 2*(g0+k) = 2*g0 + 18k (stride 18); out stride 2
            s0 = g0 * 2 + b
            nc.vector.tensor_copy(
                cls_sb[0:64, s0: s0 + 7: 2],
                ps[0:64, 2 * g0: 2 * g0 + 55: 18])
            nc.vector.tensor_copy(
                cls_sb[64:128, s0: s0 + 7: 2],
                ps[64:128, 2 * g0 + 1: 2 * g0 + 56: 18])

    # --- stage-interleaved emission: each engine's FIFO matches readiness ---
    ch0 = emit_logits(0)
    e0, s0 = emit_softmax_pre(0, ch0)
    attn0 = emit_softmax_post(0, e0, s0)
    ch1 = emit_logits(1)
    e1, s1 = emit_softmax_pre(1, ch1)
    atT0 = emit_transposes(0, attn0)
    attn1 = emit_softmax_post(1, e1, s1)
    emit_xa(0, atT0, dual=False)
    atT1 = emit_transposes(1, attn1)
    emit_xa(1, atT1, dual=True)
    emit_tail()

    # --- proj: out0[b, f] = sum_d cls[b, d] wp[d, f] + bias ---
    for c in range(2):
        ps = ps_log.tile([2, 512], dt.float32, tag=f"c{c}", name=f"proj{c}")
        for dp in range(8):
            nc.tensor.matmul(
                ps[:],
                cls_sb[:, dp * 2: dp * 2 + 2],
                wp_sb[:, dp * 1024 + c * 512: dp * 1024 + (c + 1) * 512],
                start=(dp == 0), stop=(dp == 7),
            )
        nc.vector.tensor_add(out_sb[:, c * 512:(c + 1) * 512], ps[:],
                             bp_sb[:, c * 512:(c + 1) * 512])

    nc.sync.dma_start(out_d, out_sb[:])


def _build():
    if "nc" in _cached:
        return _cached["nc"]
    from contextlib import ExitStack
    import concourse.tile as tile
    from concourse import bacc

    nc = bacc.Bacc("TRN2", target_bir_lowering=False, debug=False,
                   num_devices=NCORES)
    with tile.TileContext(nc) as tc:
        with ExitStack() as ctx:
            _kernel_body(ctx, tc)
    nc.compile()
    _cached["nc"] = nc
    return nc


def _host_prep(x, w_qkv, w_proj, b_proj):
    x = np.asarray(x, dtype=np.float32)
    w_qkv = np.asarray(w_qkv, dtype=np.float32)
    w_proj = np.asarray(w_proj, dtype=np.float32)
    b_proj = np.asarray(b_proj, dtype=np.float32)

    w_q, w_k = w_qkv[:D], w_qkv[D:2 * D]
    q0 = x[:, 0, :] @ w_q.T                                   # [B, D]
    wfold = np.einsum("bhe,hed->bhd", q0.reshape(B, H, E),
                      w_k.reshape(H, E, D)) * SCALE           # [B, H, D]
    wfT = np.ascontiguousarray(wfold.transpose(0, 2, 1))      # [B, D, H]

    xtnp = _NP_OF[XT_DTYPE]
    xnnp = _NP_OF[XN_DTYPE]
    xc = np.clip(x, -15.0, 15.0) if (xtnp is FP8 or xnnp is FP8) else x

    wv_dev = np.ascontiguousarray(w_qkv[2 * D:].T).astype(BF16)   # [d, he]
    wp_dev = np.ascontiguousarray(w_proj.T).astype(BF16)          # [d, f]
    bp_dev = np.ascontiguousarray(np.broadcast_to(b_proj, (BL, D))).astype(np.float32)
    id_dev = np.eye(48, dtype=BF16)

    in_maps = []
    for c in range(NCORES):
        b0 = c * BL
        xn = np.zeros((BL, SP, D), dtype=xnnp)
        xn[:, :S] = (x if xnnp is not FP8 else xc)[b0:b0 + BL].astype(xnnp)
        xt = np.ascontiguousarray(
            (x if xtnp is not FP8 else xc)[b0:b0 + BL].transpose(0, 2, 1)).astype(xtnp)
        wf_core = (wfT[b0:b0 + BL].reshape(BL, DT, 128, H)
                   .transpose(2, 0, 1, 3).reshape(128, BL * 128).astype(BF16))
        in_maps.append({
            "xn": xn.reshape(BL * SP, D),
            "xt": xt.reshape(BL * D, S),
            "wf": np.ascontiguousarray(wf_core),
            "wv": wv_dev,
            "wp": wp_dev,
            "bp": bp_dev,
            "ident": id_dev,
        })
    return x, in_maps


def _run(x, w_qkv, w_proj, b_proj, trace=False):
    from concourse import bass_utils
    try:
        import jax
        jax.config.update("jax_compilation_cache_dir", "/tmp/jax_pjrt_cache")
        jax.config.update("jax_persistent_cache_min_compile_time_secs", 2.0)
    except Exception:
        pass

    nc = _build()
    x, in_maps = _host_prep(x, w_qkv, w_proj, b_proj)
    res = bass_utils.run_bass_kernel_spmd(
        nc, in_maps, core_ids=list(range(NCORES)), trace=trace)

    out = x.copy()
    for c in range(NCORES):
        dev = np.asarray(res.results[c]["out"], dtype=np.float32)  # [BL, D]
        out[c * BL:(c + 1) * BL, 0, :] = dev
    return out, res


def kernel(x, w_qkv, w_proj, b_proj):
    out, _ = _run(x, w_qkv, w_proj, b_proj, trace=False)
    return out



# revision 3
# speedup vs baseline: 1.5510x; 1.5510x over previous
"""ClassAttention kernel for 8x TRN2 NeuronCores (Bass/Tile).

Problem (hardcoded): x[16, 2049, 1024], w_qkv[3072, 1024], w_proj[1024, 1024],
b_proj[1024].  Reference computes qkv projection, class-token attention
(only query position 0 attends), projection of the class token, and returns
concat([cls_tok, x[:, 1:]], axis=1).

Only output row 0 is computed; rows 1.. are x passthrough (host, mirroring the
reference's concatenate).

Algebraic restructure (same math, far fewer FLOPs):
    q0[b]        = x[b,0] @ Wq^T                       (host, tiny)
    wfold[b,h,:] = SCALE * q0[b,h,:] @ Wk_h            (host: fold q0 into Wk)
    logits[b,h,s]= sum_d x[b,s,d] * wfold[b,h,d]       (device matmul over d)
    ex           = exp(logits)                          (device, no normalize)
    xa[b,h,d]    = sum_s ex[b,h,s] * x[b,s,d]          (device matmul over s)
    sums[b,h]    = sum_s ex[b,h,s]                     (device, f32 accum)
    -- host epilogue (q0-fold-sized, O(B*D^2)): --
    attn_x       = (xa + ex_2048 * x[:,2048]) / (sums + ex_2048)
    cls[b,he]    = attn_x[b,h,:] @ Wv_h^T    (diagonal head blocks)
    out0         = cls @ Wp^T + bias

Device handles exactly s in [0, 2048) = 16 s-tiles of 128; the s=2048
remainder row is folded in on the host (it has x and wfold).

All four matmul operands (x both layouts, wfold, exp weights) are fp8e4
(e4m3) so every matmul runs in MatmulPerfMode.DoubleRow: two 128-deep
k-tiles per instruction at 2 fp8/cycle/lane - 2x PE throughput.
exp() never overflows (logits ~ N(0,1)) so no max-shift is needed; the
softmax denominator is divided out on the host, which also absorbs the
fp8-range scaling alpha folded into wfold (undone by exp's scale arg).

Sharding: data-parallel over batch, 2 batch elements per core (8 cores).
x is shipped in natural [s,d] and transposed [d,s] layouts, each
pre-permuted on the host into the exact SBUF tile layout so every DMA is
a plain linear copy with 4KB contiguous lines.
"""

import numpy as np
import ml_dtypes

BF16 = ml_dtypes.bfloat16
FP8E4 = ml_dtypes.float8_e4m3

B, S, D, H, E = 16, 2049, 1024, 16, 64
SCALE = E ** -0.5
NCORES = 8
BL = B // NCORES          # batches per core = 2
SDEV = 2048               # s rows handled on device
ST = 16                   # s-tiles of 128
DT8 = 8                   # d-tiles of 128

_cached = {}


def _kernel_body(ctx, tc):
    import concourse.bass as bass
    from concourse import mybir

    nc = tc.nc
    dt = mybir.dt
    AF = mybir.ActivationFunctionType
    DR = mybir.MatmulPerfMode.DoubleRow

    # HBM layouts (pre-permuted on host so DMAs are linear):
    #   xt row = b*256 + k8*... -> row = (b*8 + k8)*128 + p, col = s
    #   xn row = b*128 + p, col = st*1024 + d
    xt_d = nc.dram_tensor("xt", (BL * DT8 * 128, SDEV), dt.float8e4,
                          kind="ExternalInput").ap()
    xn_d = nc.dram_tensor("xn", (BL * 128, ST * 1024), dt.float8e4,
                          kind="ExternalInput").ap()
    wf_d = nc.dram_tensor("wf", (128, BL * 128), dt.float8e4,
                          kind="ExternalInput").ap()
    id_d = nc.dram_tensor("ident", (16, 16), dt.bfloat16,
                          kind="ExternalInput").ap()
    xa_d = nc.dram_tensor("xa", (16, BL * D), dt.float32,
                          kind="ExternalOutput").ap()
    se_d = nc.dram_tensor("se", (16, BL), dt.float32,
                          kind="ExternalOutput").ap()

    cpool = ctx.enter_context(tc.tile_pool(name="const", bufs=1))
    xt_pool = ctx.enter_context(tc.tile_pool(name="xt", bufs=1))
    xn_pool = ctx.enter_context(tc.tile_pool(name="xn", bufs=1))
    sm_pool = ctx.enter_context(tc.tile_pool(name="sm", bufs=1))
    st_pool = ctx.enter_context(tc.tile_pool(name="stats", bufs=2))
    at_pool = ctx.enter_context(tc.tile_pool(name="attnT", bufs=1))
    acc_pool = ctx.enter_context(tc.tile_pool(name="acc", bufs=1))

    # PSUM: logits c0..c3 (4 banks), xa (2 banks), transposes (2 banks)
    ps_log = ctx.enter_context(tc.tile_pool(name="pslog", bufs=1, space="PSUM"))
    ps_xa = ctx.enter_context(tc.tile_pool(name="psxa", bufs=1, space="PSUM"))
    ps_tr = ctx.enter_context(tc.tile_pool(name="pstr", bufs=2, space="PSUM"))

    # --- constants ---
    wf_sb = cpool.tile([128, BL * 128], dt.float8e4, tag="wf")
    nc.sync.dma_start(wf_sb[:], wf_d)
    id_sb = cpool.tile([16, 16], dt.bfloat16, tag="ident")
    nc.sync.dma_start(id_sb[:], id_d)

    # x tiles: xt[b] = [p, k8, s], xn[b] = [p, st, d]
    xt_sb = [xt_pool.tile([128, DT8, SDEV], dt.float8e4, tag=f"xt{b}",
                          name=f"xt{b}") for b in range(BL)]
    xn_sb = [xn_pool.tile([128, ST, 1024], dt.float8e4, tag=f"xn{b}",
                          name=f"xn{b}") for b in range(BL)]

    def load_xt(b, kp):
        # one d8-pair (256 d rows) for all s: 512KB, 4KB lines
        nc.sync.dma_start(
            xt_sb[b][:, kp * 2:(kp + 1) * 2, :],
            xt_d[(b * DT8 + kp * 2) * 128:(b * DT8 + (kp + 1) * 2) * 128, :]
            .rearrange("(k p) s -> p k s", p=128),
        )

    def load_xn(b, st0, st1):
        nc.sync.dma_start(
            xn_sb[b][:, st0:st1, :],
            xn_d[b * 128:(b + 1) * 128, st0 * 1024:st1 * 1024]
            .rearrange("p (st d) -> p st d", st=st1 - st0),
        )

    # persistent SBUF state
    exp_sb = [sm_pool.tile([16, SDEV], dt.bfloat16, tag=f"exp{b}",
                           name=f"exp{b}") for b in range(BL)]
    atT_sb = [at_pool.tile([128, ST, 16], dt.float8e4, tag=f"atT{b}",
                           name=f"atT{b}") for b in range(BL)]
    xa_sb = acc_pool.tile([16, BL * D], dt.float32, tag="xa")
    se_sb = acc_pool.tile([16, BL], dt.float32, tag="se")

    # --- PE warm-up: dense matmuls so the first logits run at full clock ---
    warm_sb = cpool.tile([128, 512], dt.bfloat16, tag="warm")
    nc.vector.memset(warm_sb[:], 0.0)
    for w in range(12):
        ps = ps_tr.tile([128, 512], dt.float32, tag="tr", name=f"warm{w}")
        nc.tensor.matmul(ps[:], warm_sb[:, :128], warm_sb[:], start=True,
                         stop=True)

    # --- DMA program order (= sync-queue FIFO order) ---
    # b0: xt (4 chunks), xn (4 chunks); b1: xt (4), xn (3+2 finer tail)
    for kp in range(4):
        load_xt(0, kp)
    for st0 in range(0, ST, 4):
        load_xn(0, st0, st0 + 4)
    for kp in range(4):
        load_xt(1, kp)
    for st0 in range(0, 12, 4):
        load_xn(1, st0, st0 + 4)
    load_xn(1, 12, 14)
    load_xn(1, 14, 16)

    def emit_logits(b):
        # logits[h, s] = sum_d wf[d, h] x^T[d, s]; DoubleRow over d8 pairs
        chunks = [ps_log.tile([16, 512], dt.float32, tag=f"c{c}",
                              name=f"c{c}_{b}") for c in range(4)]
        for dd in range(4):
            lhs = (wf_sb[:, b * 128 + dd * 32: b * 128 + (dd + 1) * 32]
                   .rearrange("p (two h) -> p two h", two=2))
            for c in range(4):
                nc.tensor.matmul(
                    chunks[c][:],
                    lhs,
                    xt_sb[b][:, dd * 2:(dd + 1) * 2, c * 512:(c + 1) * 512],
                    start=(dd == 0), stop=(dd == 3), perf_mode=DR,
                )
        return chunks

    def emit_exp(b, chunks):
        # logits ~ N(0,1): exp cannot overflow fp32; normalization happens on
        # the host, so emit raw exp with f32 row-sums. scale undoes ALPHA.
        sums = st_pool.tile([16, 4], dt.float32, tag="sums", name=f"sums{b}")
        for c in range(4):
            nc.scalar.activation(exp_sb[b][:, c * 512:(c + 1) * 512],
                                 chunks[c][:], AF.Exp,
                                 bias=0.0, scale=1.0 / ALPHA,
                                 accum_out=sums[:, c: c + 1])
        nc.vector.tensor_reduce(se_sb[:, b: b + 1], sums[:],
                                axis=mybir.AxisListType.X,
                                op=mybir.AluOpType.add)

    def emit_transposes(b):
        # [16,128] slices of exp -> [128,16] fp8 columns of atT, 4 per bank
        for g0 in range(0, ST, 4):
            ps = ps_tr.tile([128, 64], dt.bfloat16, tag="tr", name=f"tr{b}_{g0}")
            for k in range(4):
                st = g0 + k
                nc.tensor.transpose(ps[:, k * 16:(k + 1) * 16],
                                    exp_sb[b][:, st * 128:(st + 1) * 128],
                                    id_sb[:])
            nc.vector.tensor_copy(atT_sb[b][:, g0:g0 + 4, :],
                                  ps[:].rearrange("p (st h) -> p st h", st=4))

    def emit_xa(b):
        # xa[h, d] = sum_s ex[h,s] x[s,d]; DoubleRow over st pairs
        accs = [ps_xa.tile([16, 512], dt.float32, tag=f"xa{c}",
                           name=f"xa{c}_{b}") for c in range(2)]
        for stp in range(8):
            for c in range(2):
                nc.tensor.matmul(
                    accs[c][:],
                    atT_sb[b][:, stp * 2:(stp + 1) * 2, :],
                    xn_sb[b][:, stp * 2:(stp + 1) * 2, c * 512:(c + 1) * 512],
                    start=(stp == 0), stop=(stp == 7), perf_mode=DR,
                )
        for c in range(2):
            nc.vector.tensor_copy(
                xa_sb[:, b * D + c * 512: b * D + (c + 1) * 512], accs[c][:])

    # --- stage-interleaved emission: each engine's FIFO matches readiness ---
    ch0 = emit_logits(0)
    emit_exp(0, ch0)
    emit_transposes(0)
    emit_xa(0)
    ch1 = emit_logits(1)
    emit_exp(1, ch1)
    emit_transposes(1)
    emit_xa(1)

    nc.sync.dma_start(xa_d, xa_sb[:])
    nc.sync.dma_start(se_d, se_sb[:])


ALPHA = None  # set by _host_prep before _build


def _build():
    if "nc" in _cached:
        return _cached["nc"]
    from contextlib import ExitStack
    import concourse.tile as tile
    from concourse import bacc

    nc = bacc.Bacc("TRN2", target_bir_lowering=False, debug=False,
                   num_devices=NCORES)
    with tile.TileContext(nc) as tc:
        with ExitStack() as ctx:
            _kernel_body(ctx, tc)
    nc.compile()
    _cached["nc"] = nc
    return nc


def _host_prep(x, w_qkv, w_proj, b_proj):
    global ALPHA
    x = np.asarray(x, dtype=np.float32)
    w_qkv = np.asarray(w_qkv, dtype=np.float32)

    w_q, w_k = w_qkv[:D], w_qkv[D:2 * D]
    q0 = x[:, 0, :] @ w_q.T                                   # [B, D]
    wfold = np.einsum("bhe,hed->bhd", q0.reshape(B, H, E),
                      w_k.reshape(H, E, D)) * SCALE           # [B, H, D]
    # fp8e4 range scaling, undone by exp's scale argument on device
    ALPHA = float(2.0 ** np.floor(np.log2(64.0 / np.abs(wfold).max())))

    # wf core layout: [p, b*128 + d8*16 + h]
    wfT = np.ascontiguousarray(wfold.transpose(0, 2, 1))      # [B, D, H]
    id_dev = np.eye(16, dtype=BF16)

    in_maps = []
    for c in range(NCORES):
        b0 = c * BL
        xb = x[b0:b0 + BL, :SDEV]                             # [BL, 2048, 1024]
        # xn: [b, p, st, d]
        xn = np.ascontiguousarray(
            xb.reshape(BL, ST, 128, 1024).transpose(0, 2, 1, 3)
        ).astype(FP8E4)
        # xt: [b, k8, p, s] -> rows (b, k8, p)
        xt = np.ascontiguousarray(
            xb.transpose(0, 2, 1).reshape(BL, DT8, 128, SDEV)
            .transpose(0, 1, 2, 3)
        ).astype(FP8E4)
        wf_core = (wfT[b0:b0 + BL].reshape(BL, DT8, 128, H)
                   .transpose(2, 0, 1, 3).reshape(128, BL * 128)
                   * ALPHA).astype(FP8E4)
        in_maps.append({
            "xt": xt.reshape(BL * DT8 * 128, SDEV),
            "xn": xn.reshape(BL * 128, ST * 1024),
            "wf": np.ascontiguousarray(wf_core),
            "ident": id_dev,
        })
    return x, wfold, in_maps


def _epilogue(x, wfold, w_qkv, w_proj, b_proj, xa_all, se_all):
    """Host tail: fold s=2048, normalize, project. O(B*D^2), like the q0 fold."""
    w_v = w_qkv[2 * D:].reshape(H, E, D)
    x_last = x[:, SDEV, :]                                    # [B, D]
    l_last = np.einsum("bhd,bd->bh", wfold, x_last)           # exact f32
    e_last = np.exp(l_last)                                   # [B, H]
    xa = xa_all + e_last[:, :, None] * x_last[:, None, :]     # [B, H, D]
    sums = se_all + e_last
    attn_x = xa / sums[:, :, None]
    cls = np.einsum("bhd,hed->bhe", attn_x, w_v).reshape(B, D)
    return cls @ w_proj.T + b_proj                            # [B, D]


def _run(x, w_qkv, w_proj, b_proj, trace=False):
    from concourse import bass_utils
    try:
        import jax
        jax.config.update("jax_compilation_cache_dir", "/tmp/jax_pjrt_cache")
        jax.config.update("jax_persistent_cache_min_compile_time_secs", 2.0)
    except Exception:
        pass

    x, wfold, in_maps = _host_prep(x, w_qkv, w_proj, b_proj)
    nc = _build()
    res = bass_utils.run_bass_kernel_spmd(
        nc, in_maps, core_ids=list(range(NCORES)), trace=trace)

    xa_all = np.empty((B, H, D), np.float32)
    se_all = np.empty((B, H), np.float32)
    for c in range(NCORES):
        xa_all[c * BL:(c + 1) * BL] = np.asarray(
            res.results[c]["xa"], dtype=np.float32).reshape(
                H, BL, D).transpose(1, 0, 2)
        se_all[c * BL:(c + 1) * BL] = np.asarray(
            res.results[c]["se"], dtype=np.float32).T

    w_qkv = np.asarray(w_qkv, dtype=np.float32)
    w_proj = np.asarray(w_proj, dtype=np.float32)
    b_proj = np.asarray(b_proj, dtype=np.float32)
    out0 = _epilogue(x, wfold, w_qkv, w_proj, b_proj, xa_all, se_all)

    out = x.copy()
    out[:, 0, :] = out0
    return out, res


def kernel(x, w_qkv, w_proj, b_proj):
    out, _ = _run(x, w_qkv, w_proj, b_proj, trace=False)
    return out
